# revision 1
# baseline (speedup 1.0000x reference)
"""Causal self-attention (B=4, S=2048, D=1024, H=16, Dh=64) on 8 trn2 cores.

Sharding: data-parallel over batch (4) x tensor-parallel over head-groups (2).
Each core handles one (batch, 8-head group) shard end to end:
  qT/kT = Wq/Wk^T-slices @ xT   (transposed activations, bf16)
  v-hat = [x @ Wv-slice | 1]    (natural layout + ones column)
  wT    = exp(scores^T / 8)     (causal windows only, bf16)
  ext   = w @ v-hat             (attention out + row-sums fused)
  out^T = transpose(ext[:, :64] * 1/ext[:, 64])
  y_partial = out^T.T @ Wo-rows-slice
Host unshards: out[b] = y[2b] + y[2b+1] + out_b.
"""

import numpy as np
import ml_dtypes

import concourse.bass as bass
import concourse.mybir as mybir
import concourse.tile as tile
from concourse import bacc, bass_utils
from concourse.masks import make_identity, make_upper_triangular

F32 = mybir.dt.float32
BF16 = mybir.dt.bfloat16

S = 2048          # sequence length
D = 1024          # model dim
DH = 64           # head dim
HPC = 8           # heads per core
DV = HPC * DH     # 512: qkv slice width per core
NT = S // 128     # 16 sequence tiles
KT = D // 128     # 8 contraction tiles for QKV
VW = DH + 1       # 65: v-hat width per head (ones column appended)

# wT per-head packing: tile i covers sq in [i*128, S), width S - i*128
_WT_OFF = [0] * (NT + 1)
for _i in range(NT):
    _WT_OFF[_i + 1] = _WT_OFF[_i] + (S - _i * 128)
WT_TOTAL = _WT_OFF[NT]  # 17408


def build_nc():
    nc = bacc.Bacc("TRN2", target_bir_lowering=False, debug=False, num_devices=8)

    xT_d = nc.dram_tensor("xT", [D, S], BF16, kind="ExternalInput")
    wq_d = nc.dram_tensor("wq", [D, DV], BF16, kind="ExternalInput")
    wk_d = nc.dram_tensor("wk", [D, DV], BF16, kind="ExternalInput")
    wv_d = nc.dram_tensor("wv", [D, DV], BF16, kind="ExternalInput")
    qb_d = nc.dram_tensor("qb", [DV], F32, kind="ExternalInput")
    kb_d = nc.dram_tensor("kb", [DV], F32, kind="ExternalInput")
    vb_d = nc.dram_tensor("vb", [DV], F32, kind="ExternalInput")
    wo_d = nc.dram_tensor("wo", [DV, D], BF16, kind="ExternalInput")
    y_d = nc.dram_tensor("y", [S, D], F32, kind="ExternalOutput")

    with tile.TileContext(nc) as tc:
        with (
            tc.tile_pool(name="const", bufs=1) as const,
            tc.tile_pool(name="small", bufs=4) as small,
            tc.tile_pool(name="ysb", bufs=2) as ysbp,
            tc.tile_pool(name="scp", bufs=2, space="PSUM") as scp,
            tc.tile_pool(name="avp", bufs=3, space="PSUM") as avp,
            tc.tile_pool(name="tpp", bufs=1, space="PSUM") as tpp,
        ):
            # ---- persistent SBUF arrays ----
            ident = const.tile([128, 128], BF16)
            make_identity(nc, ident[:])
            mask = const.tile([128, 128], BF16)  # 1 where sk<=sq (r<=c)
            make_upper_triangular(nc, mask[:], val=1.0, diag=True)

            qb_sb = const.tile([128, 4], F32)
            kb_sb = const.tile([128, 4], F32)
            nc.sync.dma_start(qb_sb[:], qb_d.ap().rearrange("(m p) -> p m", p=128))
            nc.sync.dma_start(kb_sb[:], kb_d.ap().rearrange("(m p) -> p m", p=128))
            vb_row = const.tile([1, DV], F32)
            nc.sync.dma_start(vb_row[:], vb_d.ap().rearrange("(a b) -> a b", a=1))
            vb_bc = const.tile([128, DV], F32)
            nc.gpsimd.partition_broadcast(vb_bc[:], vb_row[:])

            qT = const.tile([128, 4 * S], BF16)   # part-tile m: rows dq [m*128,+128)
            kT = const.tile([128, 4 * S], BF16)
            vhat = const.tile([128, NT * VW * HPC], BF16)  # per sk-tile i: [i*520,+520)
            outT = const.tile([128, 4 * S], BF16)  # part-tile t: rows dv [t*128,+128)
            wo_sb = const.tile([128, 4 * D], BF16)
            for t in range(4):
                nc.sync.dma_start(
                    wo_sb[:, t * D:(t + 1) * D], wo_d.ap()[t * 128:(t + 1) * 128, :]
                )
            nc.vector.memset(vhat[:], 1.0)  # ones columns; v parts overwritten

            # ---- phase 1: QKV projections ----
            with (
                tc.tile_pool(name="ph1", bufs=1) as ph1,
                tc.tile_pool(name="qkvp", bufs=2, space="PSUM") as qkvp,
            ):
                xT_sb = ph1.tile([128, KT * S], BF16)
                wq_sb = ph1.tile([128, KT * DV], BF16)
                wk_sb = ph1.tile([128, KT * DV], BF16)
                wv_sb = ph1.tile([128, KT * DV], BF16)
                for k in range(KT):
                    nc.sync.dma_start(
                        xT_sb[:, k * S:(k + 1) * S], xT_d.ap()[k * 128:(k + 1) * 128, :]
                    )
                    nc.sync.dma_start(
                        wq_sb[:, k * DV:(k + 1) * DV],
                        wq_d.ap()[k * 128:(k + 1) * 128, :],
                    )
                    nc.sync.dma_start(
                        wk_sb[:, k * DV:(k + 1) * DV],
                        wk_d.ap()[k * 128:(k + 1) * 128, :],
                    )
                    nc.sync.dma_start(
                        wv_sb[:, k * DV:(k + 1) * DV],
                        wv_d.ap()[k * 128:(k + 1) * 128, :],
                    )

                # qT / kT: [dq 128-tile m, sq 512-chunk c] = W-slice^T @ xT
                for dst, wsb, bias in ((qT, wq_sb, qb_sb), (kT, wk_sb, kb_sb)):
                    for m in range(4):
                        for c in range(4):
                            ps = qkvp.tile([128, 512], F32, tag="qkvps")
                            for k in range(KT):
                                nc.tensor.matmul(
                                    ps[:],
                                    wsb[:, k * DV + m * 128:k * DV + (m + 1) * 128],
                                    xT_sb[:, k * S + c * 512:k * S + (c + 1) * 512],
                                    start=(k == 0),
                                    stop=(k == KT - 1),
                                )
                            nc.vector.tensor_scalar_add(
                                dst[:, m * S + c * 512:m * S + (c + 1) * 512],
                                ps[:],
                                bias[:, m:m + 1],
                            )
                # v natural: [s 128-tile i, dv 512] = x @ Wv-slice, + bias, into vhat
                for i in range(NT):
                    ps = qkvp.tile([128, DV], F32, tag="qkvps")
                    for k in range(KT):
                        nc.tensor.matmul(
                            ps[:],
                            xT_sb[:, k * S + i * 128:k * S + (i + 1) * 128],
                            wv_sb[:, k * DV:(k + 1) * DV],
                            start=(k == 0),
                            stop=(k == KT - 1),
                        )
                    base = i * VW * HPC
                    vdst = vhat[:, base:base + VW * HPC].rearrange(
                        "p (h w) -> p h w", w=VW
                    )[:, :, 0:DH]
                    nc.vector.tensor_tensor(
                        vdst,
                        ps[:].rearrange("p (h w) -> p h w", w=DH),
                        vb_bc[:].rearrange("p (h w) -> p h w", w=DH),
                        mybir.AluOpType.add,
                    )

            # ---- phases 2+3: per-head attention ----
            with tc.tile_pool(name="wtp", bufs=2) as wtp:
                for h in range(HPC):
                    mt = h // 2          # which 128-row part-tile of qT/kT
                    po = (h % 2) * DH    # partition offset within it
                    wT = wtp.tile([128, WT_TOTAL], BF16, tag="wT")

                    # scores^T tile i: rows sk [i*128,+128), cols sq [i*128, S)
                    for i in range(NT):
                        lhsT = kT[po:po + DH, mt * S + i * 128:mt * S + (i + 1) * 128]
                        s0 = i * 128
                        chunks = [(s0, 128)]
                        p = s0 + 128
                        while p < S:
                            n = min(512, S - p)
                            chunks.append((p, n))
                            p += n
                        for (cs, n) in chunks:
                            ps = scp.tile([128, 512], F32, tag="scps")
                            nc.tensor.matmul(
                                ps[:, 0:n],
                                lhsT,
                                qT[po:po + DH, mt * S + cs:mt * S + cs + n],
                            )
                            nc.scalar.activation(
                                wT[:, _WT_OFF[i] + cs - s0:_WT_OFF[i] + cs - s0 + n],
                                ps[:, 0:n],
                                mybir.ActivationFunctionType.Exp,
                                scale=0.125,
                            )
                        # causal mask on the diagonal 128x128 block
                        dslice = wT[:, _WT_OFF[i]:_WT_OFF[i] + 128]
                        nc.vector.tensor_tensor(
                            dslice, dslice, mask[:], mybir.AluOpType.mult
                        )

                    # ext[sq 128-tile c, 65] = w @ v-hat; col 64 = row-sum
                    for c in range(NT):
                        ext = avp.tile([128, VW], F32, tag="avps")
                        for i in range(c + 1):
                            nc.tensor.matmul(
                                ext[:],
                                wT[:, _WT_OFF[i] + (c - i) * 128:
                                     _WT_OFF[i] + (c - i + 1) * 128],
                                vhat[:, i * VW * HPC + h * VW:
                                      i * VW * HPC + (h + 1) * VW],
                                start=(i == 0),
                                stop=(i == c),
                            )
                        rinv = small.tile([128, 1], F32, tag="rinv")
                        nc.vector.reciprocal(rinv[:], ext[:, DH:DH + 1])
                        outn = small.tile([128, DH], BF16, tag="outn")
                        nc.vector.tensor_scalar_mul(outn[:], ext[:, 0:DH], rinv[:])
                        tp = tpp.tile([DH, 128], BF16, tag="tp")
                        nc.tensor.transpose(tp[:], outn[:], ident[:])
                        nc.vector.tensor_copy(
                            outT[po:po + DH, mt * S + c * 128:mt * S + (c + 1) * 128],
                            tp[:],
                        )

                # ---- phase 4: output projection (partial; host adds bias) ----
                with tc.tile_pool(name="yp", bufs=2, space="PSUM") as yp:
                    for c in range(NT):
                        ysb = ysbp.tile([128, D], F32, tag="ysb")
                        for n in range(2):
                            ps = yp.tile([128, 512], F32, tag="yps")
                            for t in range(4):
                                nc.tensor.matmul(
                                    ps[:],
                                    outT[:, t * S + c * 128:t * S + (c + 1) * 128],
                                    wo_sb[:, t * D + n * 512:t * D + (n + 1) * 512],
                                    start=(t == 0),
                                    stop=(t == 3),
                                )
                            nc.vector.tensor_copy(ysb[:, n * 512:(n + 1) * 512], ps[:])
                        nc.sync.dma_start(
                            y_d.ap()[c * 128:(c + 1) * 128, :], ysb[:]
                        )

    nc.finalize()
    return nc


_NC = None


def _get_nc():
    global _NC
    if _NC is None:
        _NC = build_nc()
    return _NC


def make_in_maps(x, qkv_w, qkv_b, out_w):
    bf = ml_dtypes.bfloat16
    x = np.asarray(x, np.float32)
    qkv_w = np.asarray(qkv_w, np.float32)
    qkv_b = np.asarray(qkv_b, np.float32)
    out_w = np.asarray(out_w, np.float32)
    in_maps = []
    for core in range(8):
        b, g = core // 2, core % 2
        hs = g * DV
        in_maps.append({
            "xT": np.ascontiguousarray(x[b].T).astype(bf),
            "wq": np.ascontiguousarray(qkv_w[:, hs:hs + DV]).astype(bf),
            "wk": np.ascontiguousarray(qkv_w[:, D + hs:D + hs + DV]).astype(bf),
            "wv": np.ascontiguousarray(qkv_w[:, 2 * D + hs:2 * D + hs + DV]).astype(bf),
            "qb": np.ascontiguousarray(qkv_b[hs:hs + DV]).astype(np.float32),
            "kb": np.ascontiguousarray(qkv_b[D + hs:D + hs + DV]).astype(np.float32),
            "vb": np.ascontiguousarray(qkv_b[2 * D + hs:2 * D + hs + DV]).astype(
                np.float32
            ),
            "wo": np.ascontiguousarray(out_w[hs:hs + DV, :]).astype(bf),
        })
    return in_maps


def run(in_maps, **kwargs):
    return bass_utils.run_bass_kernel_spmd(
        _get_nc(), in_maps, core_ids=list(range(8)), **kwargs
    )


def kernel(x, qkv_w, qkv_b, out_w, out_b):
    out_b = np.asarray(out_b, np.float32)
    res = run(make_in_maps(x, qkv_w, qkv_b, out_w))
    out = np.empty((4, S, D), np.float32)
    for b in range(4):
        out[b] = res.results[2 * b]["y"] + res.results[2 * b + 1]["y"] + out_b[None, :]
    return out


# revision 13
# speedup vs baseline: 3.6319x; 3.6319x over previous
"""Causal self-attention (B=4, S=2048, D=1024, H=16, Dh=64) on 8 trn2 cores.

Sharding: data-parallel over batch (4) x tensor-parallel over head-groups (2).
Each core handles one (batch, 8-head group) shard end to end:
  v-hat = [x @ Wv-slice | 1]    (natural layout + ones column)
  qT/kT = Wq/Wk-slice^T @ xT    (transposed activations, bf16, per head-pair)
  wT    = exp(scores^T / 8)     (causal windows only, bf16)
  ext   = w @ v-hat             (attention out + softmax row-sums fused)
  out^T = transpose(ext[:, :64] * 1/ext[:, 64])
  y_partial = out^T.T @ Wo-rows-slice
Host unshards: out[b] = y[2b] + y[2b+1] + out_b.

The emission order software-pipelines the in-order engines: each head's
exp-bound scores stream is interleaved with ready PE "filler" work (v-hat
tiles during head 0, the next pair's qT/kT chunks during odd heads, and the
previous head's w@v-hat chains), keeping TensorE busy while ScalarE drains
the exp backlog.
"""

from collections import deque

import numpy as np
import ml_dtypes

import concourse.bass as bass
import concourse.mybir as mybir
import concourse.tile as tile
from concourse import bacc, bass_utils
from concourse.masks import make_identity, make_upper_triangular

F32 = mybir.dt.float32
BF16 = mybir.dt.bfloat16

S = 2048          # sequence length
D = 1024          # model dim
DH = 64           # head dim
HPC = 8           # heads per core
DV = HPC * DH     # 512: qkv slice width per core
NT = S // 128     # 16 sequence tiles
KT = D // 128     # 8 contraction tiles for QKV
VW = DH + 1       # 65: v-hat width per head (ones column appended)

# wT per-head packing: tile i covers sq in [i*128, S), width S - i*128
_WT_OFF = [0] * (NT + 1)
for _i in range(NT):
    _WT_OFF[_i + 1] = _WT_OFF[_i] + (S - _i * 128)
WT_TOTAL = _WT_OFF[NT]  # 17408


def build_nc():
    nc = bacc.Bacc("TRN2", target_bir_lowering=False, debug=False, num_devices=8)

    xT_d = nc.dram_tensor("xT", [D, S], BF16, kind="ExternalInput")
    wq_d = nc.dram_tensor("wq", [D, DV], BF16, kind="ExternalInput")
    wk_d = nc.dram_tensor("wk", [D, DV], BF16, kind="ExternalInput")
    wv_d = nc.dram_tensor("wv", [D, DV], BF16, kind="ExternalInput")
    qb_d = nc.dram_tensor("qb", [DV], F32, kind="ExternalInput")
    kb_d = nc.dram_tensor("kb", [DV], F32, kind="ExternalInput")
    vb_d = nc.dram_tensor("vb", [DV], BF16, kind="ExternalInput")
    wo_d = nc.dram_tensor("wo", [DV, D], BF16, kind="ExternalInput")
    y_d = nc.dram_tensor("y", [S, D], F32, kind="ExternalOutput")

    with tile.TileContext(nc) as tc:
        with (
            tc.tile_pool(name="const", bufs=1) as const,
            tc.tile_pool(name="small", bufs=4) as small,
            tc.tile_pool(name="wtp", bufs=2) as wtp,
        ):
            ident = const.tile([128, 128], BF16)
            make_identity(nc, ident[:])
            mask = const.tile([128, 128], BF16)  # 1 where sk<=sq (r<=c)
            make_upper_triangular(nc, mask[:], val=1.0, diag=True)
            vhat = const.tile([128, NT * VW * HPC], BF16)  # sk-tile i at i*520
            outT = const.tile([128, 4 * S], BF16)  # part-tile t: dv [t*128,+128)
            nc.vector.memset(vhat[:], 1.0)  # ones cols; v parts overwritten

            with (
                tc.tile_pool(name="ph1", bufs=1) as ph1,
                tc.tile_pool(name="qk", bufs=2) as qkp,
                tc.tile_pool(name="scps", bufs=2, space="PSUM") as scps,
                tc.tile_pool(name="avp", bufs=2, space="PSUM") as avp,
                tc.tile_pool(name="tpp", bufs=2, space="PSUM") as tpp,
            ):
                xT_sb = ph1.tile([128, KT * S], BF16)
                wv_sb = ph1.tile([128, KT * DV], BF16)
                wq_sb = ph1.tile([128, KT * DV], BF16)
                wk_sb = ph1.tile([128, KT * DV], BF16)
                for k in range(KT):
                    nc.sync.dma_start(
                        wq_sb[:, k * DV:(k + 1) * DV],
                        wq_d.ap()[k * 128:(k + 1) * 128, :],
                    )
                    nc.sync.dma_start(
                        wk_sb[:, k * DV:(k + 1) * DV],
                        wk_d.ap()[k * 128:(k + 1) * 128, :],
                    )
                for k in range(KT):
                    nc.sync.dma_start(
                        xT_sb[:, k * S:(k + 1) * S],
                        xT_d.ap()[k * 128:(k + 1) * 128, :],
                    )
                for k in range(KT):
                    nc.sync.dma_start(
                        wv_sb[:, k * DV:(k + 1) * DV],
                        wv_d.ap()[k * 128:(k + 1) * 128, :],
                    )
                qb_sb = ph1.tile([128, 4], F32)
                kb_sb = ph1.tile([128, 4], F32)
                nc.sync.dma_start(qb_sb[:], qb_d.ap().rearrange("(m p) -> p m", p=128))
                nc.sync.dma_start(kb_sb[:], kb_d.ap().rearrange("(m p) -> p m", p=128))
                vb_row = ph1.tile([1, DV], BF16)
                nc.sync.dma_start(vb_row[:], vb_d.ap().rearrange("(a b) -> a b", a=1))
                vb_bc = ph1.tile([128, DV], BF16)
                nc.gpsimd.partition_broadcast(vb_bc[:], vb_row[:])

                # ---- PE work generators (emitted inline or as fillers) ----

                def emit_v_tile(i):
                    def emit():
                        ps = scps.tile([128, 1024], F32, tag="ps")
                        for k in range(KT):
                            nc.tensor.matmul(
                                ps[:, 0:DV],
                                xT_sb[:, k * S + i * 128:k * S + (i + 1) * 128],
                                wv_sb[:, k * DV:(k + 1) * DV],
                                start=(k == 0),
                                stop=(k == KT - 1),
                            )
                        base = i * VW * HPC
                        vdst = vhat[:, base:base + VW * HPC].rearrange(
                            "p (h w) -> p h w", w=VW
                        )[:, :, 0:DH]
                        nc.vector.tensor_tensor(
                            vdst,
                            ps[:, 0:DV].rearrange("p (h w) -> p h w", w=DH),
                            vb_bc[:].rearrange("p (h w) -> p h w", w=DH),
                            mybir.AluOpType.add,
                        )
                    return emit

                def emit_qkv_chunk(dst, wsb, bias, m, half):
                    def emit():
                        ps = scps.tile([128, 1024], F32, tag="ps")
                        for c2 in range(2):
                            col = half * 1024 + c2 * 512
                            for k in range(KT):
                                nc.tensor.matmul(
                                    ps[:, c2 * 512:(c2 + 1) * 512],
                                    wsb[:, k * DV + m * 128:k * DV + (m + 1) * 128],
                                    xT_sb[:, k * S + col:k * S + col + 512],
                                    start=(k == 0),
                                    stop=(k == KT - 1),
                                )
                        nc.vector.tensor_scalar_add(
                            dst[:, half * 1024:(half + 1) * 1024],
                            ps[:],
                            bias[:, m:m + 1],
                        )
                    return emit

                def make_qkv(m):
                    qm = qkp.tile([128, S], BF16, tag="qm")
                    km = qkp.tile([128, S], BF16, tag="km")
                    chunks = []
                    for half in range(2):
                        chunks.append(emit_qkv_chunk(qm, wq_sb, qb_sb, m, half))
                        chunks.append(emit_qkv_chunk(km, wk_sb, kb_sb, m, half))
                    return qm, km, chunks

                def emit_av_chain(h, wT, c, tpbox):
                    mt = h // 2
                    po = (h % 2) * DH

                    def emit():
                        ext = avp.tile([128, VW], F32, tag="av")
                        for i in range(c + 1):
                            nc.tensor.matmul(
                                ext[:],
                                wT[:, _WT_OFF[i] + (c - i) * 128:
                                     _WT_OFF[i] + (c - i + 1) * 128],
                                vhat[:, i * VW * HPC + h * VW:
                                      i * VW * HPC + (h + 1) * VW],
                                start=(i == 0),
                                stop=(i == c),
                            )
                        rinv = small.tile([128, 1], F32, tag="rinv")
                        nc.vector.reciprocal(rinv[:], ext[:, DH:DH + 1])
                        outn = small.tile([128, DH], BF16, tag="outn")
                        nc.vector.tensor_scalar_mul(outn[:], ext[:, 0:DH], rinv[:])
                        if c % 4 == 0:
                            tpbox["t"] = tpp.tile(
                                [DH, 512], BF16, tag="tp", name="tpt"
                            )
                        nc.tensor.transpose(
                            tpbox["t"][:, (c % 4) * 128:(c % 4 + 1) * 128],
                            outn[:], ident[:],
                        )
                        if c % 4 == 3:
                            nc.vector.tensor_copy(
                                outT[po:po + DH,
                                     mt * S + (c - 3) * 128:mt * S + (c + 1) * 128],
                                tpbox["t"][:],
                            )
                    return emit

                NPIECES = sum(
                    (S - i * 128 + 1023) // 1024 for i in range(NT)
                )  # 24

                def scores_head(h, qm, km, fill_q, wT, self_av=None):
                    """Emit head h's scoresT+exp stream, draining fill_q
                    (cost, closure) entries cost-evenly across the pieces.
                    With self_av, this head's own w@v-hat chains are emitted
                    two sk-tiles behind the exp front."""
                    po = (h % 2) * DH
                    c0 = sum(c for c, _ in fill_q)
                    done_cost = 0.0
                    pieces = 0
                    for i in range(NT):
                        w = S - i * 128
                        off = _WT_OFF[i]
                        lhsT = km[po:po + DH, i * 128:(i + 1) * 128]
                        pos = 0
                        while pos < w:
                            pw = min(1024, w - pos)
                            ps = scps.tile([128, 1024], F32, tag="ps")
                            sub = 0
                            while sub < pw:
                                n = min(512, pw - sub)
                                q0 = i * 128 + pos + sub
                                nc.tensor.matmul(
                                    ps[:, sub:sub + n], lhsT,
                                    qm[po:po + DH, q0:q0 + n],
                                )
                                sub += n
                            nc.scalar.activation(
                                wT[:, off + pos:off + pos + pw],
                                ps[:, 0:pw],
                                mybir.ActivationFunctionType.Exp,
                                scale=0.125,
                            )
                            pos += pw
                            pieces += 1
                            target = c0 * pieces / NPIECES
                            while done_cost < target and fill_q:
                                cost, emit = fill_q.popleft()
                                emit()
                                done_cost += cost
                        dslice = wT[:, off:off + 128]
                        nc.vector.tensor_tensor(
                            dslice, dslice, mask[:], mybir.AluOpType.mult
                        )
                        if self_av is not None and i >= 2:
                            self_av(i - 2)
                    while fill_q:
                        cost, emit = fill_q.popleft()
                        emit()
                    if self_av is not None:
                        self_av(NT - 2)
                        self_av(NT - 1)

                # ---- pipelined emission ----
                V_COST, QKV_COST = 1.7, 3.4
                AV_COST = lambda c: 0.3 + 0.03 * (c + 1)  # noqa: E731
                fill_q = deque()
                qm0, km0, chunks0 = make_qkv(0)
                for ch in chunks0:   # prologue: first pair's qT/kT
                    ch()
                cur_qk = (qm0, km0)
                nxt_qk = None
                pending_qkv = []

                for h in range(HPC):
                    m = h // 2
                    if h == 0:
                        fill_q.extend(
                            (V_COST, emit_v_tile(i)) for i in range(NT)
                        )
                    if h % 2 == 0 and m < 3:
                        qmn, kmn, chn = make_qkv(m + 1)
                        nxt_qk = (qmn, kmn)
                        fill_q.extendleft(
                            (QKV_COST, ch) for ch in reversed(chn[:2])
                        )
                        pending_qkv = chn[2:]
                    if h % 2 == 1:
                        fill_q.extendleft(
                            (QKV_COST, ch) for ch in reversed(pending_qkv)
                        )
                        pending_qkv = []
                    qm, km = cur_qk
                    wT = wtp.tile([128, WT_TOTAL], BF16, tag="wT", name="wTt")
                    tpbox = {}
                    if h == HPC - 1:
                        scores_head(
                            h, qm, km, fill_q, wT,
                            self_av=lambda c: emit_av_chain(h, wT, c, tpbox)(),
                        )
                    else:
                        scores_head(h, qm, km, fill_q, wT)
                        fill_q.extend(
                            (AV_COST(c), emit_av_chain(h, wT, c, tpbox))
                            for c in range(NT)
                        )
                    if h % 2 == 1:
                        cur_qk = nxt_qk
                # any remaining fillers
                while fill_q:
                    fill_q.popleft()[1]()

            # ---- tail: output projection (partial; host adds bias) ----
            with (
                tc.tile_pool(name="tail", bufs=1) as tailp,
                tc.tile_pool(name="ysb", bufs=2) as ysbp,
                tc.tile_pool(name="yp", bufs=2, space="PSUM") as yp,
            ):
                wo_sb = tailp.tile([128, 4 * D], BF16)
                for t in range(4):
                    nc.sync.dma_start(
                        wo_sb[:, t * D:(t + 1) * D],
                        wo_d.ap()[t * 128:(t + 1) * 128, :],
                    )
                for c in range(NT):
                    yps = yp.tile([128, D], F32, tag="yps")
                    for n in range(2):
                        for t in range(4):
                            nc.tensor.matmul(
                                yps[:, n * 512:(n + 1) * 512],
                                outT[:, t * S + c * 128:t * S + (c + 1) * 128],
                                wo_sb[:, t * D + n * 512:t * D + (n + 1) * 512],
                                start=(t == 0),
                                stop=(t == 3),
                            )
                    ysb = ysbp.tile([128, D], F32, tag="ysb")
                    nc.vector.tensor_copy(ysb[:], yps[:])
                    nc.sync.dma_start(y_d.ap()[c * 128:(c + 1) * 128, :], ysb[:])

    nc.finalize()
    return nc


_NC = None


def _get_nc():
    global _NC
    if _NC is None:
        _NC = build_nc()
    return _NC


def make_in_maps(x, qkv_w, qkv_b, out_w):
    bf = ml_dtypes.bfloat16
    x = np.asarray(x, np.float32)
    qkv_w = np.asarray(qkv_w, np.float32)
    qkv_b = np.asarray(qkv_b, np.float32)
    out_w = np.asarray(out_w, np.float32)
    in_maps = []
    for core in range(8):
        b, g = core // 2, core % 2
        hs = g * DV
        in_maps.append({
            "xT": np.ascontiguousarray(x[b].T).astype(bf),
            "wq": np.ascontiguousarray(qkv_w[:, hs:hs + DV]).astype(bf),
            "wk": np.ascontiguousarray(qkv_w[:, D + hs:D + hs + DV]).astype(bf),
            "wv": np.ascontiguousarray(qkv_w[:, 2 * D + hs:2 * D + hs + DV]).astype(bf),
            "qb": np.ascontiguousarray(qkv_b[hs:hs + DV]).astype(np.float32),
            "kb": np.ascontiguousarray(qkv_b[D + hs:D + hs + DV]).astype(np.float32),
            "vb": np.ascontiguousarray(qkv_b[2 * D + hs:2 * D + hs + DV]).astype(bf),
            "wo": np.ascontiguousarray(out_w[hs:hs + DV, :]).astype(bf),
        })
    return in_maps


def run(in_maps, **kwargs):
    return bass_utils.run_bass_kernel_spmd(
        _get_nc(), in_maps, core_ids=list(range(8)), **kwargs
    )


def kernel(x, qkv_w, qkv_b, out_w, out_b):
    out_b = np.asarray(out_b, np.float32)
    res = run(make_in_maps(x, qkv_w, qkv_b, out_w))
    out = np.empty((4, S, D), np.float32)
    for b in range(4):
        out[b] = res.results[2 * b]["y"] + res.results[2 * b + 1]["y"] + out_b[None, :]
    return out
